# revision 22
# baseline (speedup 1.0000x reference)
"""EnsembleActor MLP kernel for Trainium2 (Bass/Tile), expert-parallel over 8 cores.

Math per ensemble head e (E=8, B=4096, OBS=256, H=1024, A=64):
    h1 = relu(x @ W1 + b1)
    h2 = relu(h1 @ W2 + b2)
    mu = h2 @ W3 + b3
    Gs = sum(|mu|, axis=-1)/A ; g = max(Gs, 1)
    mu = mu / g ; pi = mu + 0.1*noise
    return tanh(mu), tanh(pi)

Sharding: one head per NeuronCore (8 heads, 8 cores). Same program on all
cores; per-core inputs differ. No collectives.

Layout: activations flow feature-major ([feat, batch]) through ALL layers so
weights are always the PE-stationary operand in natural [K, M] layout.
All matmuls are bf16 in / fp32 PSUM accumulate (fp32-wide moving operands
stream at ~1.6 cyc/col on the PE; bf16 streams at 1.0 — and walrus rejects
mixed f32r/bf16 operand pairs, so weights are bf16 too).
The per-row epilogue runs feature-major too: Gs comes from a ones-vector
matmul over |mu|, 1/g is broadcast back across partitions with a rank-1
matmul, so no transposes anywhere on-device. Host supplies x/noise
pre-transposed and re-transposes the [A, B] outputs (cheap numpy).
"""

import os
import sys

import numpy as np

for _p in ("/opt/trn_rl_repo", os.path.expanduser("~/.axon_site/_ro/trn_rl_repo")):
    if os.path.isdir(_p) and _p not in sys.path:
        sys.path.insert(0, _p)

E, B, OBS, H, A = 8, 4096, 256, 1024, 64
ACT_NOISE = 0.1
P = 128          # SBUF/PSUM partitions
BT = 512         # batch tile (matmul moving free dim; one PSUM bank fp32)
NBT = B // BT    # 8 batch tiles
KO = OBS // P    # 2 k-chunks in layer 1
KH = H // P     # 8 k-chunks in layers 2/3

_PROGRAM = None  # compiled Bacc program cache (one per process)


def _build_program():
    from contextlib import ExitStack

    import concourse.bass as bass
    import concourse.tile as tile
    from concourse import bacc, mybir

    f32 = mybir.dt.float32
    f32r = mybir.dt.float32r
    bf16 = mybir.dt.bfloat16
    FT = mybir.ActivationFunctionType
    OP = mybir.AluOpType

    nc = bacc.Bacc("TRN2", target_bir_lowering=False, debug=False)

    xT = nc.dram_tensor("xTbf", [OBS, B], bf16, kind="ExternalInput").ap()
    nzT = nc.dram_tensor("nzT", [A, B], f32, kind="ExternalInput").ap()
    W1 = nc.dram_tensor("W1", [OBS, H], bf16, kind="ExternalInput").ap()
    W2 = nc.dram_tensor("W2", [H, H], bf16, kind="ExternalInput").ap()
    W3 = nc.dram_tensor("W3bf", [H, A], bf16, kind="ExternalInput").ap()
    b1 = nc.dram_tensor("b1c", [P, KH], f32, kind="ExternalInput").ap()
    b2 = nc.dram_tensor("b2c", [P, KH], f32, kind="ExternalInput").ap()
    b3 = nc.dram_tensor("b3col", [A, 1], f32, kind="ExternalInput").ap()
    onesA = nc.dram_tensor("onesA", [A, 1], bf16, kind="ExternalInput").ap()
    ones1 = nc.dram_tensor("ones1", [1, A], bf16, kind="ExternalInput").ap()
    muT_out = nc.dram_tensor("muT", [A, B], f32, kind="ExternalOutput").ap()
    piT_out = nc.dram_tensor("piT", [A, B], f32, kind="ExternalOutput").ap()

    with tile.TileContext(nc) as tc, ExitStack() as ctx:
        wpool = ctx.enter_context(tc.tile_pool(name="weights", bufs=1))
        xpool = ctx.enter_context(tc.tile_pool(name="x", bufs=3))
        hpool = ctx.enter_context(tc.tile_pool(name="h", bufs=4))
        epool = ctx.enter_context(tc.tile_pool(name="epi", bufs=4))
        pspool = ctx.enter_context(tc.tile_pool(name="ps", bufs=6, space="PSUM"))
        fmpool = ctx.enter_context(tc.tile_pool(name="fm", bufs=1, space="PSUM"))
        grpool = ctx.enter_context(tc.tile_pool(name="gr", bufs=1, space="PSUM"))

        # ---- persistent weights/constants in SBUF ----
        # Queue plan (both sync and scalar are fast HWDGE rings): sync carries
        # W1 + x/noise/output traffic; scalar carries biases + W3; W2 is split
        # across both so layer 2's weights land before layer 2 starts.
        w1s = []
        for k in range(KO):
            t = wpool.tile([P, H], bf16, name=f"w1_{k}", tag=f"w1_{k}")
            # halve time-to-first-matmul: each W1 chunk split across queues
            nc.sync.dma_start(out=t[:, :H // 2], in_=W1[k * P:(k + 1) * P, :H // 2])
            nc.scalar.dma_start(out=t[:, H // 2:], in_=W1[k * P:(k + 1) * P, H // 2:])
            w1s.append(t)

        def load_x(bt, split=False):
            bsl = bass.ds(bt * BT, BT)
            xts = []
            for k in range(KO):
                t = xpool.tile([P, BT], bf16, name=f"xt{k}", tag=f"xt{k}")
                if split:
                    nc.sync.dma_start(
                        out=t[:, :BT // 2],
                        in_=xT[k * P:(k + 1) * P, bass.ds(bt * BT, BT // 2)])
                    nc.scalar.dma_start(
                        out=t[:, BT // 2:],
                        in_=xT[k * P:(k + 1) * P, bass.ds(bt * BT + BT // 2, BT // 2)])
                else:
                    nc.sync.dma_start(out=t[:], in_=xT[k * P:(k + 1) * P, bsl])
                xts.append(t)
            return xts

        # PE warm-up: dummy matmuls on a memset scratch tile keep the PE busy
        # while the first weight/x DMAs stream in, so the HAM clock-gate is
        # already at full rate when the real layer-1 matmuls start. Results go
        # to a scratch PSUM row nothing reads.
        scratch = wpool.tile([P, BT], bf16, name="scratch", tag="scratch")
        nc.vector.memset(scratch[:], 0.0)
        for _w in range(24):
            dps = grpool.tile([1, BT], f32, name="dps", tag="gr")
            nc.tensor.matmul(dps[:], lhsT=scratch[:, 0:1], rhs=scratch[:],
                             start=True, stop=True)

        xts0 = load_x(0, split=True)
        xts1 = load_x(1, split=True)
        xts2 = load_x(2, split=True)

        b1s = wpool.tile([P, KH], f32, name="b1s", tag="b1s")
        nc.scalar.dma_start(out=b1s[:], in_=b1[:, :])
        b2s = wpool.tile([P, KH], f32, name="b2s", tag="b2s")
        nc.scalar.dma_start(out=b2s[:], in_=b2[:, :])
        b3s = wpool.tile([A, 1], f32, name="b3s", tag="b3s")
        nc.scalar.dma_start(out=b3s[:], in_=b3[:, :])
        onesAs = wpool.tile([A, 1], bf16, name="onesAs", tag="onesAs")
        nc.scalar.dma_start(out=onesAs[:], in_=onesA[:, :])
        ones1s = wpool.tile([1, A], bf16, name="ones1s", tag="ones1s")
        nc.scalar.dma_start(out=ones1s[:], in_=ones1[:, :])
        w2s = []
        for k in range(KH):
            t = wpool.tile([P, H], bf16, name=f"w2_{k}", tag=f"w2_{k}")
            eng = nc.sync if k % 2 == 0 else nc.scalar
            eng.dma_start(out=t[:], in_=W2[k * P:(k + 1) * P, :])
            w2s.append(t)
        w3s = wpool.tile([P, KH, A], bf16, name="w3s", tag="w3s")
        nc.scalar.dma_start(out=w3s[:], in_=W3.rearrange("(k p) a -> p k a", k=KH, p=P))

        def layer1(xts):
            h1s = []
            for oc in range(KH):
                ps = pspool.tile([P, BT], f32, name="ps1", tag="ps")
                for k in range(KO):
                    nc.tensor.matmul(
                        ps[:],
                        lhsT=w1s[k][:, oc * P:(oc + 1) * P],
                        rhs=xts[k][:],
                        start=(k == 0),
                        stop=(k == KO - 1),
                    )
                h = hpool.tile([P, BT], bf16, name=f"h1_{oc}", tag=f"h1_{oc}")
                if oc % 2 == 0:
                    nc.vector.tensor_scalar(
                        out=h[:], in0=ps[:],
                        scalar1=b1s[:, oc:oc + 1], scalar2=0.0,
                        op0=OP.add, op1=OP.max,
                    )
                else:
                    nc.scalar.activation(
                        out=h[:], in_=ps[:], func=FT.Relu,
                        bias=b1s[:, oc:oc + 1],
                    )
                h1s.append(h)
            return h1s

        def layer2(h1s):
            h2s = []
            for oc in range(KH):
                ps = pspool.tile([P, BT], f32, name="ps2", tag="ps")
                for k in range(KH):
                    nc.tensor.matmul(
                        ps[:],
                        lhsT=w2s[k][:, oc * P:(oc + 1) * P],
                        rhs=h1s[k][:],
                        start=(k == 0),
                        stop=(k == KH - 1),
                    )
                h = hpool.tile([P, BT], bf16, name=f"h2_{oc}", tag=f"h2_{oc}")
                if oc % 2 == 0:
                    nc.vector.tensor_scalar(
                        out=h[:], in0=ps[:],
                        scalar1=b2s[:, oc:oc + 1], scalar2=0.0,
                        op0=OP.add, op1=OP.max,
                    )
                else:
                    nc.scalar.activation(
                        out=h[:], in_=ps[:], func=FT.Relu,
                        bias=b2s[:, oc:oc + 1],
                    )
                h2s.append(h)
            return h2s

        def layer3(bt, h2s):
            bsl = bass.ds(bt * BT, BT)
            nzt = xpool.tile([A, BT], f32, name="nzt", tag="nzt")
            nc.sync.dma_start(out=nzt[:], in_=nzT[:, bsl])
            fm = fmpool.tile([A, BT], f32, name="fm", tag="fm")
            for k in range(KH):
                nc.tensor.matmul(
                    fm[:],
                    lhsT=w3s[:, k, :],
                    rhs=h2s[k][:],
                    start=(k == 0),
                    stop=(k == KH - 1),
                )
            # mu = fm + b3 (per-partition bias); |mu| in bf16 feeds the Gs
            # ones-matmul next round.
            mu_sb = epool.tile([A, BT], f32, name="mu_sb", tag="mu_sb")
            nc.scalar.activation(
                out=mu_sb[:], in_=fm[:], func=FT.Identity, bias=b3s[:, 0:1],
            )
            amu = epool.tile([A, BT], bf16, name="amu", tag="amu")
            nc.scalar.activation(
                out=amu[:], in_=fm[:], func=FT.Abs, bias=b3s[:, 0:1],
            )
            return {"bt": bt, "mu_sb": mu_sb, "amu": amu, "nzt": nzt}

        def epi_stage1(pv, c0, cw):
            # Gs row-reduction: gs[1, b] = sum_a |mu[a, b]|  (ones-vector matmul)
            csl = bass.ds(c0 - pv.get("coff", 0), cw)
            pool, ptag = pv.get("pspool", (grpool, "gr"))
            gs = pool.tile([1, cw], f32, name="gs", tag=ptag)
            nc.tensor.matmul(gs[:], lhsT=onesAs[:], rhs=pv["amu"][:, csl],
                             start=True, stop=True)
            g = epool.tile([1, cw], f32, name="g", tag="g")
            nc.vector.tensor_scalar(
                out=g[:], in0=gs[:], scalar1=1.0 / A, scalar2=1.0,
                op0=OP.mult, op1=OP.max,
            )
            gbf = epool.tile([1, cw], bf16, name="gbf", tag="gbf")
            with nc.allow_low_precision(reason="1/g is 1.0 exactly for almost all rows"):
                nc.vector.reciprocal(out=gbf[:], in_=g[:])
            pv[f"gbf{c0}"] = gbf

        def epi_stage2(pv, c0, cw):
            # broadcast 1/g across the A partitions via rank-1 matmul, then
            # mu = tanh(mu/g), pi = tanh(mu/g + 0.1*noise)
            csl = bass.ds(c0 - pv.get("coff", 0), cw)
            nsl = bass.ds(c0, cw)
            osl = bass.ds(pv["bt"] * BT + c0, cw)
            pool, ptag = pv.get("pspool", (grpool, "gr"))
            rb = pool.tile([A, cw], f32, name="rb", tag=ptag)
            nc.tensor.matmul(rb[:], lhsT=ones1s[:], rhs=pv[f"gbf{c0}"][:],
                             start=True, stop=True)
            mu_n = epool.tile([A, cw], f32, name="mu_n", tag="mu_n")
            nc.vector.tensor_tensor(out=mu_n[:], in0=pv["mu_sb"][:, csl],
                                    in1=rb[:], op=OP.mult)
            muT_sb = epool.tile([A, cw], f32, name="muT_sb", tag="muT_sb")
            nc.scalar.activation(out=muT_sb[:], in_=mu_n[:], func=FT.Tanh)
            nc.sync.dma_start(out=muT_out[:, osl], in_=muT_sb[:])
            pi_pre = epool.tile([A, cw], f32, name="pi_pre", tag="pi_pre")
            nc.vector.tensor_tensor(out=pi_pre[:], in0=mu_n[:],
                                    in1=pv["nzt"][:, nsl], op=OP.add)
            piT_sb = epool.tile([A, cw], f32, name="piT_sb", tag="piT_sb")
            nc.scalar.activation(out=piT_sb[:], in_=pi_pre[:], func=FT.Tanh)
            nc.sync.dma_start(out=piT_out[:, osl], in_=piT_sb[:])

        # Software pipeline: layer 1 runs one batch-tile ahead of layers 2/3
        # (keeps PE fed while W2 streams in at the start, and takes layer 1's
        # relu drain off the PE critical path); the scale/tanh epilogue runs
        # one batch-tile behind so its DVE/ACT chain never stalls the PE.
        def layer3_flush(bt, h2s):
            # Last tile: nothing follows to hide the epilogue chain under, so
            # run layer 3 + epilogue in column chunks, software-pipelined, to
            # keep the exposed tail down to one chunk's latency.
            bsl = bass.ds(bt * BT, BT)
            nzt = xpool.tile([A, BT], f32, name="nzt", tag="nzt")
            nc.sync.dma_start(out=nzt[:], in_=nzT[:, bsl])
            FC = 4
            cw = BT // FC
            pvs = []
            for j in range(FC):
                c0 = j * cw
                csl = bass.ds(c0, cw)
                fm = pspool.tile([A, cw], f32, name="fmc", tag="ps")
                for k in range(KH):
                    nc.tensor.matmul(
                        fm[:],
                        lhsT=w3s[:, k, :],
                        rhs=h2s[k][:, csl],
                        start=(k == 0),
                        stop=(k == KH - 1),
                    )
                mu_sb = epool.tile([A, cw], f32, name="mu_sb", tag="mu_sb")
                nc.scalar.activation(
                    out=mu_sb[:], in_=fm[:], func=FT.Identity, bias=b3s[:, 0:1],
                )
                amu = epool.tile([A, cw], bf16, name="amu", tag="amu")
                nc.scalar.activation(
                    out=amu[:], in_=fm[:], func=FT.Abs, bias=b3s[:, 0:1],
                )
                pvs.append({"bt": bt, "mu_sb": mu_sb, "amu": amu, "nzt": nzt,
                            "coff": c0, "pspool": (pspool, "ps")})
                if j >= 1:
                    epi_stage1(pvs[j - 1], (j - 1) * cw, cw)
                if j >= 2:
                    epi_stage2(pvs[j - 2], (j - 2) * cw, cw)
            epi_stage1(pvs[FC - 1], (FC - 1) * cw, cw)
            epi_stage2(pvs[FC - 2], (FC - 2) * cw, cw)
            epi_stage2(pvs[FC - 1], (FC - 1) * cw, cw)

        # Layer 1 runs TWO batch-tiles ahead: enough queued PE work that W2's
        # initial DMA stream finishes before layer 2 first needs it.
        h1q = [layer1(xts0), layer1(xts1)]
        xt_pre = {2: xts2}
        prev = None
        for bt in range(NBT):
            nxt = bt + 2
            if nxt < NBT:
                xts = xt_pre.pop(nxt, None) or load_x(nxt)
                h1q.append(layer1(xts))
            if prev is not None:
                epi_stage1(prev, 0, BT)
            h2s = layer2(h1q.pop(0))
            if prev is not None:
                epi_stage2(prev, 0, BT)
            if bt < NBT - 1:
                prev = layer3(bt, h2s)
            else:
                layer3_flush(bt, h2s)

    nc.compile()
    return nc


def _get_program():
    global _PROGRAM
    if _PROGRAM is None:
        _PROGRAM = _build_program()
    return _PROGRAM


def run(inputs, trace=False, trace_cores=None, tmpdir=None):
    """Returns (outputs_tuple, BassKernelResults)."""
    import ml_dtypes

    from concourse.bass_utils import run_bass_kernel_spmd

    nc = _get_program()
    bf = ml_dtypes.bfloat16

    x = np.asarray(inputs["x"], dtype=np.float32)
    noise = np.asarray(inputs["noise"], dtype=np.float32)
    W1 = np.asarray(inputs["W1"], dtype=np.float32)
    b1 = np.asarray(inputs["b1"], dtype=np.float32)
    W2 = np.asarray(inputs["W2"], dtype=np.float32)
    b2 = np.asarray(inputs["b2"], dtype=np.float32)
    W3 = np.asarray(inputs["W3"], dtype=np.float32)
    b3 = np.asarray(inputs["b3"], dtype=np.float32)

    in_maps = []
    for e in range(E):
        in_maps.append({
            "xTbf": np.ascontiguousarray(x[e].T.astype(bf)),
            "nzT": np.ascontiguousarray((ACT_NOISE * noise[e]).T),
            "W1": np.ascontiguousarray(W1[e].astype(bf)),
            "W2": np.ascontiguousarray(W2[e].astype(bf)),
            "W3bf": W3[e].astype(bf),
            "b1c": np.ascontiguousarray(b1[e].reshape(KH, P).T),
            "b2c": np.ascontiguousarray(b2[e].reshape(KH, P).T),
            "b3col": b3[e].reshape(A, 1),
            "onesA": np.ones((A, 1), dtype=bf),
            "ones1": np.ones((1, A), dtype=bf),
        })

    res = run_bass_kernel_spmd(
        nc, in_maps, core_ids=list(range(E)), trace=trace,
        trace_cores=trace_cores, tmpdir=tmpdir,
    )
    mu = np.stack([res.results[e]["muT"].T for e in range(E)])
    pi = np.stack([res.results[e]["piT"].T for e in range(E)])
    return (np.ascontiguousarray(mu), np.ascontiguousarray(pi)), res


def kernel(**inputs):
    outs, _ = run(inputs, trace=False)
    return outs


# revision 23
# speedup vs baseline: 1.0255x; 1.0255x over previous
"""EnsembleActor MLP kernel for Trainium2 (Bass/Tile), expert-parallel over 8 cores.

Math per ensemble head e (E=8, B=4096, OBS=256, H=1024, A=64):
    h1 = relu(x @ W1 + b1)
    h2 = relu(h1 @ W2 + b2)
    mu = h2 @ W3 + b3
    Gs = sum(|mu|, axis=-1)/A ; g = max(Gs, 1)
    mu = mu / g ; pi = mu + 0.1*noise
    return tanh(mu), tanh(pi)

Sharding: one head per NeuronCore (8 heads, 8 cores). Same program on all
cores; per-core inputs differ. No collectives.

Layout: activations flow feature-major ([feat, batch]) through ALL layers so
weights are always the PE-stationary operand in natural [K, M] layout.
All matmuls are bf16 in / fp32 PSUM accumulate (fp32-wide moving operands
stream at ~1.6 cyc/col on the PE; bf16 streams at 1.0 — and walrus rejects
mixed f32r/bf16 operand pairs, so weights are bf16 too).
The per-row epilogue runs feature-major too: Gs comes from a ones-vector
matmul over |mu|, 1/g is broadcast back across partitions with a rank-1
matmul, so no transposes anywhere on-device. Host supplies x/noise
pre-transposed and re-transposes the [A, B] outputs (cheap numpy).
"""

import os
import sys

import numpy as np

for _p in ("/opt/trn_rl_repo", os.path.expanduser("~/.axon_site/_ro/trn_rl_repo")):
    if os.path.isdir(_p) and _p not in sys.path:
        sys.path.insert(0, _p)

E, B, OBS, H, A = 8, 4096, 256, 1024, 64
ACT_NOISE = 0.1
P = 128          # SBUF/PSUM partitions
BT = 512         # batch tile (matmul moving free dim; one PSUM bank fp32)
NBT = B // BT    # 8 batch tiles
KO = OBS // P    # 2 k-chunks in layer 1
KH = H // P     # 8 k-chunks in layers 2/3

_PROGRAM = None  # compiled Bacc program cache (one per process)


def _build_program():
    from contextlib import ExitStack

    import concourse.bass as bass
    import concourse.tile as tile
    from concourse import bacc, mybir

    f32 = mybir.dt.float32
    f32r = mybir.dt.float32r
    bf16 = mybir.dt.bfloat16
    FT = mybir.ActivationFunctionType
    OP = mybir.AluOpType

    nc = bacc.Bacc("TRN2", target_bir_lowering=False, debug=False)

    xT = nc.dram_tensor("xTbf", [OBS, B], bf16, kind="ExternalInput").ap()
    nzT = nc.dram_tensor("nzT", [A, B], f32, kind="ExternalInput").ap()
    W1 = nc.dram_tensor("W1", [OBS, H], bf16, kind="ExternalInput").ap()
    W2 = nc.dram_tensor("W2", [H, H], bf16, kind="ExternalInput").ap()
    W3 = nc.dram_tensor("W3bf", [H, A], bf16, kind="ExternalInput").ap()
    b1 = nc.dram_tensor("b1c", [P, KH], f32, kind="ExternalInput").ap()
    b2 = nc.dram_tensor("b2c", [P, KH], f32, kind="ExternalInput").ap()
    b3 = nc.dram_tensor("b3col", [A, 1], f32, kind="ExternalInput").ap()
    onesA = nc.dram_tensor("onesA", [A, 1], bf16, kind="ExternalInput").ap()
    ones1 = nc.dram_tensor("ones1", [1, A], bf16, kind="ExternalInput").ap()
    muT_out = nc.dram_tensor("muT", [A, B], f32, kind="ExternalOutput").ap()
    piT_out = nc.dram_tensor("piT", [A, B], f32, kind="ExternalOutput").ap()

    with tile.TileContext(nc) as tc, ExitStack() as ctx:
        wpool = ctx.enter_context(tc.tile_pool(name="weights", bufs=1))
        xpool = ctx.enter_context(tc.tile_pool(name="x", bufs=3))
        hpool = ctx.enter_context(tc.tile_pool(name="h", bufs=4))
        epool = ctx.enter_context(tc.tile_pool(name="epi", bufs=4))
        pspool = ctx.enter_context(tc.tile_pool(name="ps", bufs=6, space="PSUM"))
        fmpool = ctx.enter_context(tc.tile_pool(name="fm", bufs=1, space="PSUM"))
        grpool = ctx.enter_context(tc.tile_pool(name="gr", bufs=1, space="PSUM"))

        # ---- persistent weights/constants in SBUF ----
        # Queue plan (both sync and scalar are fast HWDGE rings): sync carries
        # W1 + x/noise/output traffic; scalar carries biases + W3; W2 is split
        # across both so layer 2's weights land before layer 2 starts.
        w1s = []
        for k in range(KO):
            t = wpool.tile([P, H], bf16, name=f"w1_{k}", tag=f"w1_{k}")
            # halve time-to-first-matmul: each W1 chunk split across queues
            nc.sync.dma_start(out=t[:, :H // 2], in_=W1[k * P:(k + 1) * P, :H // 2])
            nc.scalar.dma_start(out=t[:, H // 2:], in_=W1[k * P:(k + 1) * P, H // 2:])
            w1s.append(t)

        def load_x(bt, split=False):
            bsl = bass.ds(bt * BT, BT)
            xts = []
            for k in range(KO):
                t = xpool.tile([P, BT], bf16, name=f"xt{k}", tag=f"xt{k}")
                if split:
                    nc.sync.dma_start(
                        out=t[:, :BT // 2],
                        in_=xT[k * P:(k + 1) * P, bass.ds(bt * BT, BT // 2)])
                    nc.scalar.dma_start(
                        out=t[:, BT // 2:],
                        in_=xT[k * P:(k + 1) * P, bass.ds(bt * BT + BT // 2, BT // 2)])
                else:
                    nc.sync.dma_start(out=t[:], in_=xT[k * P:(k + 1) * P, bsl])
                xts.append(t)
            return xts

        xts0 = load_x(0, split=True)
        xts1 = load_x(1)
        xts2 = load_x(2)

        b1s = wpool.tile([P, KH], f32, name="b1s", tag="b1s")
        nc.scalar.dma_start(out=b1s[:], in_=b1[:, :])
        b2s = wpool.tile([P, KH], f32, name="b2s", tag="b2s")
        nc.scalar.dma_start(out=b2s[:], in_=b2[:, :])
        b3s = wpool.tile([A, 1], f32, name="b3s", tag="b3s")
        nc.scalar.dma_start(out=b3s[:], in_=b3[:, :])
        onesAs = wpool.tile([A, 1], bf16, name="onesAs", tag="onesAs")
        nc.scalar.dma_start(out=onesAs[:], in_=onesA[:, :])
        ones1s = wpool.tile([1, A], bf16, name="ones1s", tag="ones1s")
        nc.scalar.dma_start(out=ones1s[:], in_=ones1[:, :])
        w2s = []
        for k in range(KH):
            t = wpool.tile([P, H], bf16, name=f"w2_{k}", tag=f"w2_{k}")
            eng = nc.sync if k % 2 == 0 else nc.scalar
            eng.dma_start(out=t[:], in_=W2[k * P:(k + 1) * P, :])
            w2s.append(t)
        w3s = wpool.tile([P, KH, A], bf16, name="w3s", tag="w3s")
        nc.scalar.dma_start(out=w3s[:], in_=W3.rearrange("(k p) a -> p k a", k=KH, p=P))

        def layer1(xts):
            h1s = []
            for oc in range(KH):
                ps = pspool.tile([P, BT], f32, name="ps1", tag="ps")
                for k in range(KO):
                    nc.tensor.matmul(
                        ps[:],
                        lhsT=w1s[k][:, oc * P:(oc + 1) * P],
                        rhs=xts[k][:],
                        start=(k == 0),
                        stop=(k == KO - 1),
                    )
                h = hpool.tile([P, BT], bf16, name=f"h1_{oc}", tag=f"h1_{oc}")
                if oc % 2 == 0:
                    nc.vector.tensor_scalar(
                        out=h[:], in0=ps[:],
                        scalar1=b1s[:, oc:oc + 1], scalar2=0.0,
                        op0=OP.add, op1=OP.max,
                    )
                else:
                    nc.scalar.activation(
                        out=h[:], in_=ps[:], func=FT.Relu,
                        bias=b1s[:, oc:oc + 1],
                    )
                h1s.append(h)
            return h1s

        def layer2(h1s):
            h2s = []
            for oc in range(KH):
                ps = pspool.tile([P, BT], f32, name="ps2", tag="ps")
                for k in range(KH):
                    nc.tensor.matmul(
                        ps[:],
                        lhsT=w2s[k][:, oc * P:(oc + 1) * P],
                        rhs=h1s[k][:],
                        start=(k == 0),
                        stop=(k == KH - 1),
                    )
                h = hpool.tile([P, BT], bf16, name=f"h2_{oc}", tag=f"h2_{oc}")
                if oc % 2 == 0:
                    nc.vector.tensor_scalar(
                        out=h[:], in0=ps[:],
                        scalar1=b2s[:, oc:oc + 1], scalar2=0.0,
                        op0=OP.add, op1=OP.max,
                    )
                else:
                    nc.scalar.activation(
                        out=h[:], in_=ps[:], func=FT.Relu,
                        bias=b2s[:, oc:oc + 1],
                    )
                h2s.append(h)
            return h2s

        def layer3(bt, h2s):
            bsl = bass.ds(bt * BT, BT)
            nzt = xpool.tile([A, BT], f32, name="nzt", tag="nzt")
            nc.sync.dma_start(out=nzt[:], in_=nzT[:, bsl])
            fm = fmpool.tile([A, BT], f32, name="fm", tag="fm")
            for k in range(KH):
                nc.tensor.matmul(
                    fm[:],
                    lhsT=w3s[:, k, :],
                    rhs=h2s[k][:],
                    start=(k == 0),
                    stop=(k == KH - 1),
                )
            # mu = fm + b3 (per-partition bias); |mu| in bf16 feeds the Gs
            # ones-matmul next round.
            mu_sb = epool.tile([A, BT], f32, name="mu_sb", tag="mu_sb")
            nc.scalar.activation(
                out=mu_sb[:], in_=fm[:], func=FT.Identity, bias=b3s[:, 0:1],
            )
            amu = epool.tile([A, BT], bf16, name="amu", tag="amu")
            nc.scalar.activation(
                out=amu[:], in_=fm[:], func=FT.Abs, bias=b3s[:, 0:1],
            )
            return {"bt": bt, "mu_sb": mu_sb, "amu": amu, "nzt": nzt}

        def epi_stage1(pv, c0, cw):
            # Gs row-reduction: gs[1, b] = sum_a |mu[a, b]|  (ones-vector matmul)
            csl = bass.ds(c0 - pv.get("coff", 0), cw)
            pool, ptag = pv.get("pspool", (grpool, "gr"))
            gs = pool.tile([1, cw], f32, name="gs", tag=ptag)
            nc.tensor.matmul(gs[:], lhsT=onesAs[:], rhs=pv["amu"][:, csl],
                             start=True, stop=True)
            g = epool.tile([1, cw], f32, name="g", tag="g")
            nc.vector.tensor_scalar(
                out=g[:], in0=gs[:], scalar1=1.0 / A, scalar2=1.0,
                op0=OP.mult, op1=OP.max,
            )
            gbf = epool.tile([1, cw], bf16, name="gbf", tag="gbf")
            with nc.allow_low_precision(reason="1/g is 1.0 exactly for almost all rows"):
                nc.vector.reciprocal(out=gbf[:], in_=g[:])
            pv[f"gbf{c0}"] = gbf

        def epi_stage2(pv, c0, cw):
            # broadcast 1/g across the A partitions via rank-1 matmul, then
            # mu = tanh(mu/g), pi = tanh(mu/g + 0.1*noise)
            csl = bass.ds(c0 - pv.get("coff", 0), cw)
            nsl = bass.ds(c0, cw)
            osl = bass.ds(pv["bt"] * BT + c0, cw)
            pool, ptag = pv.get("pspool", (grpool, "gr"))
            rb = pool.tile([A, cw], f32, name="rb", tag=ptag)
            nc.tensor.matmul(rb[:], lhsT=ones1s[:], rhs=pv[f"gbf{c0}"][:],
                             start=True, stop=True)
            mu_n = epool.tile([A, cw], f32, name="mu_n", tag="mu_n")
            nc.vector.tensor_tensor(out=mu_n[:], in0=pv["mu_sb"][:, csl],
                                    in1=rb[:], op=OP.mult)
            muT_sb = epool.tile([A, cw], f32, name="muT_sb", tag="muT_sb")
            nc.scalar.activation(out=muT_sb[:], in_=mu_n[:], func=FT.Tanh)
            nc.sync.dma_start(out=muT_out[:, osl], in_=muT_sb[:])
            pi_pre = epool.tile([A, cw], f32, name="pi_pre", tag="pi_pre")
            nc.vector.tensor_tensor(out=pi_pre[:], in0=mu_n[:],
                                    in1=pv["nzt"][:, nsl], op=OP.add)
            piT_sb = epool.tile([A, cw], f32, name="piT_sb", tag="piT_sb")
            nc.scalar.activation(out=piT_sb[:], in_=pi_pre[:], func=FT.Tanh)
            nc.sync.dma_start(out=piT_out[:, osl], in_=piT_sb[:])

        # Software pipeline: layer 1 runs one batch-tile ahead of layers 2/3
        # (keeps PE fed while W2 streams in at the start, and takes layer 1's
        # relu drain off the PE critical path); the scale/tanh epilogue runs
        # one batch-tile behind so its DVE/ACT chain never stalls the PE.
        def layer3_flush(bt, h2s):
            # Last tile: nothing follows to hide the epilogue chain under, so
            # run layer 3 + epilogue in column chunks, software-pipelined, to
            # keep the exposed tail down to one chunk's latency.
            bsl = bass.ds(bt * BT, BT)
            nzt = xpool.tile([A, BT], f32, name="nzt", tag="nzt")
            nc.sync.dma_start(out=nzt[:], in_=nzT[:, bsl])
            FC = 4
            cw = BT // FC
            pvs = []
            for j in range(FC):
                c0 = j * cw
                csl = bass.ds(c0, cw)
                fm = pspool.tile([A, cw], f32, name="fmc", tag="ps")
                for k in range(KH):
                    nc.tensor.matmul(
                        fm[:],
                        lhsT=w3s[:, k, :],
                        rhs=h2s[k][:, csl],
                        start=(k == 0),
                        stop=(k == KH - 1),
                    )
                mu_sb = epool.tile([A, cw], f32, name="mu_sb", tag="mu_sb")
                nc.scalar.activation(
                    out=mu_sb[:], in_=fm[:], func=FT.Identity, bias=b3s[:, 0:1],
                )
                amu = epool.tile([A, cw], bf16, name="amu", tag="amu")
                nc.scalar.activation(
                    out=amu[:], in_=fm[:], func=FT.Abs, bias=b3s[:, 0:1],
                )
                pvs.append({"bt": bt, "mu_sb": mu_sb, "amu": amu, "nzt": nzt,
                            "coff": c0, "pspool": (pspool, "ps")})
                if j >= 1:
                    epi_stage1(pvs[j - 1], (j - 1) * cw, cw)
                if j >= 2:
                    epi_stage2(pvs[j - 2], (j - 2) * cw, cw)
            epi_stage1(pvs[FC - 1], (FC - 1) * cw, cw)
            epi_stage2(pvs[FC - 2], (FC - 2) * cw, cw)
            epi_stage2(pvs[FC - 1], (FC - 1) * cw, cw)

        # Layer 1 runs TWO batch-tiles ahead: enough queued PE work that W2's
        # initial DMA stream finishes before layer 2 first needs it.
        h1q = [layer1(xts0), layer1(xts1)]
        xt_pre = {2: xts2}
        prev = None
        for bt in range(NBT):
            nxt = bt + 2
            if nxt < NBT:
                xts = xt_pre.pop(nxt, None) or load_x(nxt)
                h1q.append(layer1(xts))
            if prev is not None:
                epi_stage1(prev, 0, BT)
            h2s = layer2(h1q.pop(0))
            if prev is not None:
                epi_stage2(prev, 0, BT)
            if bt < NBT - 1:
                prev = layer3(bt, h2s)
            else:
                layer3_flush(bt, h2s)

    nc.compile()
    return nc


def _get_program():
    global _PROGRAM
    if _PROGRAM is None:
        _PROGRAM = _build_program()
    return _PROGRAM


def run(inputs, trace=False, trace_cores=None, tmpdir=None):
    """Returns (outputs_tuple, BassKernelResults)."""
    import ml_dtypes

    from concourse.bass_utils import run_bass_kernel_spmd

    nc = _get_program()
    bf = ml_dtypes.bfloat16

    x = np.asarray(inputs["x"], dtype=np.float32)
    noise = np.asarray(inputs["noise"], dtype=np.float32)
    W1 = np.asarray(inputs["W1"], dtype=np.float32)
    b1 = np.asarray(inputs["b1"], dtype=np.float32)
    W2 = np.asarray(inputs["W2"], dtype=np.float32)
    b2 = np.asarray(inputs["b2"], dtype=np.float32)
    W3 = np.asarray(inputs["W3"], dtype=np.float32)
    b3 = np.asarray(inputs["b3"], dtype=np.float32)

    in_maps = []
    for e in range(E):
        in_maps.append({
            "xTbf": np.ascontiguousarray(x[e].T.astype(bf)),
            "nzT": np.ascontiguousarray((ACT_NOISE * noise[e]).T),
            "W1": np.ascontiguousarray(W1[e].astype(bf)),
            "W2": np.ascontiguousarray(W2[e].astype(bf)),
            "W3bf": W3[e].astype(bf),
            "b1c": np.ascontiguousarray(b1[e].reshape(KH, P).T),
            "b2c": np.ascontiguousarray(b2[e].reshape(KH, P).T),
            "b3col": b3[e].reshape(A, 1),
            "onesA": np.ones((A, 1), dtype=bf),
            "ones1": np.ones((1, A), dtype=bf),
        })

    res = run_bass_kernel_spmd(
        nc, in_maps, core_ids=list(range(E)), trace=trace,
        trace_cores=trace_cores, tmpdir=tmpdir,
    )
    mu = np.stack([res.results[e]["muT"].T for e in range(E)])
    pi = np.stack([res.results[e]["piT"].T for e in range(E)])
    return (np.ascontiguousarray(mu), np.ascontiguousarray(pi)), res


def kernel(**inputs):
    outs, _ = run(inputs, trace=False)
    return outs
